# revision 6
# baseline (speedup 1.0000x reference)
"""AdaptiveGCNLayer Trainium2 kernel (8 NeuronCores, data-parallel over frames).

The reference module's adaptive-adjacency branch is dead code (its result is
never used).  Because edge_index is shared by every frame (offsets just shift
it per frame), the live computation collapses to

    out[f] = M @ x[f] @ gcn_W + gcn_b        for every frame f

with a single 25x25 normalized-adjacency matrix M (PyG GCNConv norm with
self-loops) computed on host from the 48 edges.

Sharding: frames are data-parallel across the 8 cores.  Each core's shard is
packed on host into tile-major layout [125 partitions, 205 tiles, 128 ch]
(5 frames = 125 rows per tile; the ragged tail is zero-padded) so every
HBM<->SBUF DMA is per-partition contiguous.

Device kernel (per core):
  - mm1: T1 = lhsT(x_tile).T @ (I5 (x) M^T)   -> (M5 @ X)^T in PSUM (no transposes)
  - copy T1 -> SBUF
  - mm2: O = lhsT(T1).T @ W                   -> natural row-major output in PSUM
  - DVE adds bias while copying PSUM -> SBUF
  - big contiguous HWDGE DMAs in and out

Two compute modes (KERNEL_MODE env): "bf16" casts x to bf16 on ACT/DVE and
runs bf16 matmuls; "f32r" feeds fp32 bits straight to the PE as float32r
with the moving free dim padded to 256 (full-rate per the cost model).
"""

import os
import numpy as np
import ml_dtypes

B, V, C = 8192, 25, 128
NCORES = 8
FRAMES_PER_CORE = B // NCORES          # 1024
ROWS = FRAMES_PER_CORE * V             # 25600
FPT = 5                                # frames per matmul tile
TROWS = FPT * V                        # 125 rows per tile
NT = 205                               # tiles per core (last one padded)
FULL_T = ROWS // TROWS                 # 204 full tiles
TAIL_ROWS = ROWS - FULL_T * TROWS      # 100
TPG = 41                               # tiles per DMA group
NGROUPS = NT // TPG                    # 5
JB = 4                                 # tiles per PSUM batch
MODE = os.environ.get("KERNEL_MODE", "bf16")
NF32R = 256                            # padded moving free dim for f32r

_CACHE = {}


def _build_graph(mode=MODE):
    import concourse.mybir as mybir
    import concourse.tile as tile
    from concourse import bacc

    f32 = mybir.dt.float32
    f32r = mybir.dt.float32r
    bf16 = mybir.dt.bfloat16

    nc = bacc.Bacc("TRN2", target_bir_lowering=False, debug=False,
                   num_devices=NCORES)

    NMM = NF32R if mode == "f32r" else C
    cdt = f32 if mode == "f32r" else bf16

    x_in = nc.declare_dram_parameter("x", [TROWS, NT, C], f32, isOutput=False)
    m5t_in = nc.declare_dram_parameter("m5t", [TROWS, NMM], cdt, isOutput=False)
    w_in = nc.declare_dram_parameter("w", [C, NMM], cdt, isOutput=False)
    b_in = nc.declare_dram_parameter("bias", [TROWS, JB, C], f32, isOutput=False)
    out_ext = nc.declare_dram_parameter("out", [TROWS, NT, C], f32, isOutput=True)

    # input slices within a group: DMA (+ cast) pipeline at this grain
    SL = [(0, 11), (11, 10), (21, 10), (31, 10)]

    def mmv(ap):  # matmul operand view
        return ap.bitcast(f32r) if mode == "f32r" else ap

    with tile.TileContext(nc) as tc:
        with (
            tc.tile_pool(name="consts", bufs=1) as consts,
            tc.tile_pool(name="xf32", bufs=2) as xf_pool,
            tc.tile_pool(name="xp", bufs=2) as xp,
            tc.tile_pool(name="op", bufs=2) as op_pool,
            tc.tile_pool(name="t1s", bufs=3) as t1sp,
            tc.tile_pool(name="t1psum", bufs=2, space=tile.bass.MemorySpace.PSUM) as t1pp,
            tc.tile_pool(name="opsum", bufs=2, space=tile.bass.MemorySpace.PSUM) as opp,
        ):
            m5t_sb = consts.tile([TROWS, NMM], cdt)
            w_sb = consts.tile([C, NMM], cdt)
            bias_sb = consts.tile([TROWS, JB, C], f32)
            nc.sync.dma_start(out=m5t_sb[:], in_=m5t_in[:])
            nc.sync.dma_start(out=w_sb[:], in_=w_in[:])
            nc.sync.dma_start(out=bias_sb[:], in_=b_in[:])

            for g in range(NGROUPS):
                t0 = g * TPG
                x_f = xf_pool.tile([128, TPG, C], f32, tag="xf")
                if mode == "bf16":
                    x_t = xp.tile([128, TPG, C], bf16, tag="x")
                else:
                    x_t = x_f
                for si, (s0, sn) in enumerate(SL):
                    # fast HWDGE load (contiguous per partition)
                    nc.sync.dma_start(out=x_f[0:TROWS, s0:s0 + sn, :],
                                      in_=x_in[:, t0 + s0:t0 + s0 + sn, :])
                    if mode == "bf16":
                        # f32 -> bf16 cast split between ACT and DVE
                        if si % 2 == 0:
                            nc.scalar.copy(x_t[0:TROWS, s0:s0 + sn, :],
                                           x_f[0:TROWS, s0:s0 + sn, :])
                        else:
                            nc.vector.tensor_copy(x_t[0:TROWS, s0:s0 + sn, :],
                                                  x_f[0:TROWS, s0:s0 + sn, :])

                o_t = op_pool.tile([128, TPG, C], f32, tag="o")

                for j0 in range(0, TPG, JB):
                    nb = min(JB, TPG - j0)
                    t1p = t1pp.tile([128, JB, NMM], f32, tag="t1p")
                    for u in range(nb):
                        nc.tensor.matmul(t1p[:, u, :],
                                         lhsT=mmv(x_t[0:TROWS, j0 + u, :]),
                                         rhs=mmv(m5t_sb[:, :]),
                                         start=True, stop=True)
                    t1s = t1sp.tile([128, JB, C], cdt, tag="t1s")
                    nc.scalar.copy(t1s[:, 0:nb, :], t1p[:, 0:nb, 0:C])
                    o_ps = opp.tile([128, JB, NMM], f32, tag="ops")
                    for u in range(nb):
                        nc.tensor.matmul(o_ps[:, u, :],
                                         lhsT=mmv(t1s[:, u, :]),
                                         rhs=mmv(w_sb[:, :]),
                                         start=True, stop=True)
                    nc.vector.tensor_add(o_t[0:TROWS, j0:j0 + nb, :],
                                         o_ps[0:TROWS, 0:nb, 0:C],
                                         bias_sb[:, 0:nb, :])

                nc.sync.dma_start(out=out_ext[:, t0:t0 + TPG, :],
                                  in_=o_t[0:TROWS, :, :])

    nc.compile()
    return nc


def _get_graph():
    if "nc" not in _CACHE:
        _CACHE["nc"] = _build_graph()
    return _CACHE["nc"]


def _host_prep(edge_index, gcn_W, gcn_b, mode=MODE):
    ei = np.asarray(edge_index).astype(np.int64)
    rows, cols = ei[0], ei[1]
    deg = np.bincount(cols, minlength=V).astype(np.float32) + 1.0  # + self loop
    dis = (1.0 / np.sqrt(deg)).astype(np.float32)
    M = np.zeros((V, V), np.float32)
    np.add.at(M, (cols, rows), dis[rows] * dis[cols])
    M[np.arange(V), np.arange(V)] += dis * dis
    nmm = NF32R if mode == "f32r" else C
    hdt = np.float32 if mode == "f32r" else ml_dtypes.bfloat16
    m5t_pad = np.zeros((TROWS, nmm), np.float32)
    m5t_pad[:, :TROWS] = np.kron(np.eye(FPT, dtype=np.float32), M.T)
    w_pad = np.zeros((C, nmm), np.float32)
    w_pad[:, :C] = np.asarray(gcn_W, np.float32)
    bias_t = np.ascontiguousarray(
        np.broadcast_to(np.asarray(gcn_b, np.float32), (TROWS, JB, C)))
    return m5t_pad.astype(hdt), w_pad.astype(hdt), bias_t


def _pack(x):
    """(B, V, C) f32 -> per-core tile-major [NCORES, TROWS, NT, C]."""
    xr = np.asarray(x, np.float32).reshape(NCORES, ROWS, C)
    packed = np.zeros((NCORES, NT, TROWS, C), np.float32)
    packed[:, :FULL_T] = xr[:, :FULL_T * TROWS].reshape(NCORES, FULL_T, TROWS, C)
    packed[:, FULL_T, :TAIL_ROWS] = xr[:, FULL_T * TROWS:]
    return np.ascontiguousarray(packed.transpose(0, 2, 1, 3))


def _unpack(outs):
    """[NCORES, TROWS, NT, C] -> (B, V, C)."""
    o = outs.transpose(0, 2, 1, 3)  # [NCORES, NT, TROWS, C]
    res = np.empty((NCORES, ROWS, C), np.float32)
    res[:, :FULL_T * TROWS] = o[:, :FULL_T].reshape(NCORES, FULL_T * TROWS, C)
    res[:, FULL_T * TROWS:] = o[:, FULL_T, :TAIL_ROWS]
    return res.reshape(B, V, C)


def kernel(x, edge_index, adj_matrix=None, aw_W=None, aw_b=None,
           gcn_W=None, gcn_b=None, **_unused):
    from concourse.bass_utils import run_bass_kernel_spmd

    m5t_h, w_h, bias_t = _host_prep(edge_index, gcn_W, gcn_b)
    xp = _pack(x)
    in_maps = [{"x": xp[i], "m5t": m5t_h, "w": w_h, "bias": bias_t}
               for i in range(NCORES)]
    res = run_bass_kernel_spmd(_get_graph(), in_maps,
                               core_ids=list(range(NCORES)))
    out = np.stack([r["out"] for r in res.results])
    return _unpack(out)


# revision 8
# speedup vs baseline: 1.6532x; 1.6532x over previous
"""AdaptiveGCNLayer Trainium2 kernel (8 NeuronCores, data-parallel over frames).

The reference module's adaptive-adjacency branch is dead code (its result is
never used).  Because edge_index is shared by every frame (offsets just shift
it per frame), the live computation collapses to

    out[f] = M @ x[f] @ gcn_W + gcn_b        for every frame f

with a single 25x25 normalized-adjacency matrix M (PyG GCNConv norm with
self-loops) computed on host from the 48 edges.

Sharding: frames are data-parallel across the 8 cores.  Each core's shard is
packed on host into tile-major layout [125 partitions, 205 tiles, 128 ch]
(5 frames = 125 rows per tile; the ragged tail is zero-padded) so every
HBM<->SBUF DMA is per-partition contiguous.

Device kernel (per core):
  - mm1: T1 = lhsT(x_tile).T @ (I5 (x) M^T)   -> (M5 @ X)^T in PSUM (no transposes)
  - copy T1 -> SBUF
  - mm2: O = lhsT(T1).T @ W                   -> natural row-major output in PSUM
  - DVE adds bias while copying PSUM -> SBUF
  - big contiguous HWDGE DMAs in and out

Two compute modes (KERNEL_MODE env): "bf16" casts x to bf16 on ACT/DVE and
runs bf16 matmuls; "f32r" feeds fp32 bits straight to the PE as float32r
with the moving free dim padded to 256 (full-rate per the cost model).
"""

import os
import numpy as np
import ml_dtypes

B, V, C = 8192, 25, 128
NCORES = 8
FRAMES_PER_CORE = B // NCORES          # 1024
ROWS = FRAMES_PER_CORE * V             # 25600
FPT = 5                                # frames per matmul tile
TROWS = FPT * V                        # 125 rows per tile
NT = 205                               # tiles per core (last one padded)
FULL_T = ROWS // TROWS                 # 204 full tiles
TAIL_ROWS = ROWS - FULL_T * TROWS      # 100
TPG = 41                               # tiles per DMA group
NGROUPS = NT // TPG                    # 5
JB = 4                                 # tiles per PSUM batch
MODE = os.environ.get("KERNEL_MODE", "bf16")
NF32R = 256                            # padded moving free dim for f32r

_CACHE = {}


def _build_graph(mode=MODE):
    import concourse.mybir as mybir
    import concourse.tile as tile
    from concourse import bacc

    f32 = mybir.dt.float32
    f32r = mybir.dt.float32r
    bf16 = mybir.dt.bfloat16

    nc = bacc.Bacc("TRN2", target_bir_lowering=False, debug=False,
                   num_devices=NCORES)

    # x arrives pre-cast to bf16 by the host (halves input traffic, no
    # on-chip cast stage needed)
    x_in = nc.declare_dram_parameter("x", [TROWS, NT, C], bf16, isOutput=False)
    m5t_in = nc.declare_dram_parameter("m5t", [TROWS, C], bf16, isOutput=False)
    w_in = nc.declare_dram_parameter("w", [C, C], bf16, isOutput=False)
    b_in = nc.declare_dram_parameter("bias", [TROWS, JB, C], f32, isOutput=False)
    out_ext = nc.declare_dram_parameter("out", [TROWS, NT, C], f32, isOutput=True)

    with tile.TileContext(nc) as tc:
        with (
            tc.tile_pool(name="consts", bufs=1) as consts,
            tc.tile_pool(name="xp", bufs=3) as xp,
            tc.tile_pool(name="op", bufs=2) as op_pool,
            tc.tile_pool(name="t1s", bufs=3) as t1sp,
            tc.tile_pool(name="t1psum", bufs=2, space=tile.bass.MemorySpace.PSUM) as t1pp,
            tc.tile_pool(name="opsum", bufs=2, space=tile.bass.MemorySpace.PSUM) as opp,
        ):
            m5t_sb = consts.tile([TROWS, C], bf16)
            w_sb = consts.tile([C, C], bf16)
            bias_sb = consts.tile([TROWS, JB, C], f32)
            nc.sync.dma_start(out=m5t_sb[:], in_=m5t_in[:])
            nc.sync.dma_start(out=w_sb[:], in_=w_in[:])
            nc.sync.dma_start(out=bias_sb[:], in_=b_in[:])

            for g in range(NGROUPS):
                t0 = g * TPG
                x_t = xp.tile([128, TPG, C], bf16, tag="x")
                # HWDGE load, contiguous per partition, bf16
                nc.sync.dma_start(out=x_t[0:TROWS, :, :],
                                  in_=x_in[:, t0:t0 + TPG, :])

                o_t = op_pool.tile([128, TPG, C], f32, tag="o")

                for j0 in range(0, TPG, JB):
                    nb = min(JB, TPG - j0)
                    t1p = t1pp.tile([128, JB, C], f32, tag="t1p")
                    for u in range(nb):
                        nc.tensor.matmul(t1p[:, u, :],
                                         lhsT=x_t[0:TROWS, j0 + u, :],
                                         rhs=m5t_sb[:, :],
                                         start=True, stop=True)
                    t1s = t1sp.tile([128, JB, C], bf16, tag="t1s")
                    nc.scalar.copy(t1s[:, 0:nb, :], t1p[:, 0:nb, :])
                    o_ps = opp.tile([128, JB, C], f32, tag="ops")
                    for u in range(nb):
                        nc.tensor.matmul(o_ps[:, u, :],
                                         lhsT=t1s[:, u, :],
                                         rhs=w_sb[:, :],
                                         start=True, stop=True)
                    nc.vector.tensor_add(o_t[0:TROWS, j0:j0 + nb, :],
                                         o_ps[0:TROWS, 0:nb, :],
                                         bias_sb[:, 0:nb, :])

                # output on the SWDGE path so input/output streams ride
                # different DMA queues
                nc.gpsimd.dma_start(out=out_ext[:, t0:t0 + TPG, :],
                                    in_=o_t[0:TROWS, :, :])

    nc.compile()
    return nc


def _get_graph():
    if "nc" not in _CACHE:
        _CACHE["nc"] = _build_graph()
    return _CACHE["nc"]


def _host_prep(edge_index, gcn_W, gcn_b, mode=MODE):
    ei = np.asarray(edge_index).astype(np.int64)
    rows, cols = ei[0], ei[1]
    deg = np.bincount(cols, minlength=V).astype(np.float32) + 1.0  # + self loop
    dis = (1.0 / np.sqrt(deg)).astype(np.float32)
    M = np.zeros((V, V), np.float32)
    np.add.at(M, (cols, rows), dis[rows] * dis[cols])
    M[np.arange(V), np.arange(V)] += dis * dis
    m5t_pad = np.zeros((TROWS, C), np.float32)
    m5t_pad[:, :TROWS] = np.kron(np.eye(FPT, dtype=np.float32), M.T)
    bias_t = np.ascontiguousarray(
        np.broadcast_to(np.asarray(gcn_b, np.float32), (TROWS, JB, C)))
    return (m5t_pad.astype(ml_dtypes.bfloat16),
            np.asarray(gcn_W, np.float32).astype(ml_dtypes.bfloat16),
            bias_t)


def _pack(x):
    """(B, V, C) f32 -> per-core tile-major bf16 [NCORES, TROWS, NT, C]."""
    xr = np.asarray(x, np.float32).reshape(NCORES, ROWS, C)
    packed = np.zeros((NCORES, NT, TROWS, C), np.float32)
    packed[:, :FULL_T] = xr[:, :FULL_T * TROWS].reshape(NCORES, FULL_T, TROWS, C)
    packed[:, FULL_T, :TAIL_ROWS] = xr[:, FULL_T * TROWS:]
    return np.ascontiguousarray(
        packed.transpose(0, 2, 1, 3).astype(ml_dtypes.bfloat16))


def _unpack(outs):
    """[NCORES, TROWS, NT, C] -> (B, V, C)."""
    o = outs.transpose(0, 2, 1, 3)  # [NCORES, NT, TROWS, C]
    res = np.empty((NCORES, ROWS, C), np.float32)
    res[:, :FULL_T * TROWS] = o[:, :FULL_T].reshape(NCORES, FULL_T * TROWS, C)
    res[:, FULL_T * TROWS:] = o[:, FULL_T, :TAIL_ROWS]
    return res.reshape(B, V, C)


def kernel(x, edge_index, adj_matrix=None, aw_W=None, aw_b=None,
           gcn_W=None, gcn_b=None, **_unused):
    from concourse.bass_utils import run_bass_kernel_spmd

    m5t_h, w_h, bias_t = _host_prep(edge_index, gcn_W, gcn_b)
    xp = _pack(x)
    in_maps = [{"x": xp[i], "m5t": m5t_h, "w": w_h, "bias": bias_t}
               for i in range(NCORES)]
    res = run_bass_kernel_spmd(_get_graph(), in_maps,
                               core_ids=list(range(NCORES)))
    out = np.stack([r["out"] for r in res.results])
    return _unpack(out)


# revision 11
# speedup vs baseline: 1.9972x; 1.2081x over previous
"""AdaptiveGCNLayer Trainium2 kernel (8 NeuronCores, data-parallel over frames).

The reference module's adaptive-adjacency branch is dead code (its result is
never used).  Because edge_index is shared by every frame (offsets just shift
it per frame), the live computation collapses to

    out[f] = M @ x[f] @ gcn_W + gcn_b        for every frame f

with a single 25x25 normalized-adjacency matrix M (PyG GCNConv norm with
self-loops) computed on host from the 48 edges.

Sharding: frames are data-parallel across the 8 cores.  Each core's shard is
packed on host into tile-major layout [125 partitions, 205 tiles, 128 ch]
(5 frames = 125 rows per tile; the ragged tail is zero-padded) so every
HBM<->SBUF DMA is per-partition contiguous.

Device kernel (per core):
  - mm1: T1 = lhsT(x_tile).T @ (I5 (x) M^T)   -> (M5 @ X)^T in PSUM (no transposes)
  - copy T1 -> SBUF
  - mm2: O = lhsT(T1).T @ W                   -> natural row-major output in PSUM
  - DVE adds bias while copying PSUM -> SBUF
  - big contiguous HWDGE DMAs in and out

Two compute modes (KERNEL_MODE env): "bf16" casts x to bf16 on ACT/DVE and
runs bf16 matmuls; "f32r" feeds fp32 bits straight to the PE as float32r
with the moving free dim padded to 256 (full-rate per the cost model).
"""

import os
import numpy as np
import ml_dtypes

B, V, C = 8192, 25, 128
NCORES = 8
FRAMES_PER_CORE = B // NCORES          # 1024
ROWS = FRAMES_PER_CORE * V             # 25600
FPT = 5                                # frames per matmul tile
TROWS = FPT * V                        # 125 rows per tile
NT = 205                               # tiles per core (last one padded)
FULL_T = ROWS // TROWS                 # 204 full tiles
TAIL_ROWS = ROWS - FULL_T * TROWS      # 100
TPG = 41                               # tiles per DMA group
NGROUPS = NT // TPG                    # 5
JB = 4                                 # tiles per PSUM batch
MODE = os.environ.get("KERNEL_MODE", "bf16")
NF32R = 256                            # padded moving free dim for f32r

_CACHE = {}


def _build_graph(mode=MODE):
    import concourse.mybir as mybir
    import concourse.tile as tile
    from concourse import bacc

    f32 = mybir.dt.float32
    f32r = mybir.dt.float32r
    bf16 = mybir.dt.bfloat16

    nc = bacc.Bacc("TRN2", target_bir_lowering=False, debug=False,
                   num_devices=NCORES)

    # x arrives pre-cast to bf16 by the host (halves input traffic, no
    # on-chip cast stage needed)
    x_in = nc.declare_dram_parameter("x", [TROWS, NT, C], bf16, isOutput=False)
    m5t_in = nc.declare_dram_parameter("m5t", [TROWS, C], bf16, isOutput=False)
    w_in = nc.declare_dram_parameter("w", [C, C], bf16, isOutput=False)
    b_in = nc.declare_dram_parameter("bias", [TROWS, JB, C], f32, isOutput=False)
    out_ext = nc.declare_dram_parameter("out", [TROWS, NT, C], f32, isOutput=True)

    with tile.TileContext(nc) as tc:
        with (
            tc.tile_pool(name="consts", bufs=1) as consts,
            tc.tile_pool(name="xp", bufs=5) as xp,
            tc.tile_pool(name="op", bufs=2) as op_pool,
            tc.tile_pool(name="t1s", bufs=3) as t1sp,
            tc.tile_pool(name="t1psum", bufs=2, space=tile.bass.MemorySpace.PSUM) as t1pp,
            tc.tile_pool(name="opsum", bufs=2, space=tile.bass.MemorySpace.PSUM) as opp,
        ):
            m5t_sb = consts.tile([TROWS, C], bf16)
            w_sb = consts.tile([C, C], bf16)
            bias_sb = consts.tile([TROWS, JB, C], f32)
            nc.sync.dma_start(out=m5t_sb[:], in_=m5t_in[:])
            nc.sync.dma_start(out=w_sb[:], in_=w_in[:])
            nc.sync.dma_start(out=bias_sb[:], in_=b_in[:])

            for g in range(NGROUPS):
                t0 = g * TPG
                x_t = xp.tile([128, TPG, C], bf16, tag="x")
                # HWDGE loads, contiguous per partition, bf16; sliced so
                # the first matmuls unblock before the whole group lands
                for s0, sn in ((0, 14), (14, 14), (28, 13)):
                    nc.sync.dma_start(out=x_t[0:TROWS, s0:s0 + sn, :],
                                      in_=x_in[:, t0 + s0:t0 + s0 + sn, :])

                o_t = op_pool.tile([128, TPG, C], f32, tag="o")

                for j0 in range(0, TPG, JB):
                    nb = min(JB, TPG - j0)
                    t1p = t1pp.tile([128, JB, C], f32, tag="t1p")
                    for u in range(nb):
                        nc.tensor.matmul(t1p[:, u, :],
                                         lhsT=x_t[0:TROWS, j0 + u, :],
                                         rhs=m5t_sb[:, :],
                                         start=True, stop=True)
                    t1s = t1sp.tile([128, JB, C], bf16, tag="t1s")
                    nc.scalar.copy(t1s[:, 0:nb, :], t1p[:, 0:nb, :])
                    o_ps = opp.tile([128, JB, C], f32, tag="ops")
                    for u in range(nb):
                        nc.tensor.matmul(o_ps[:, u, :],
                                         lhsT=t1s[:, u, :],
                                         rhs=w_sb[:, :],
                                         start=True, stop=True)
                    nc.vector.tensor_add(o_t[0:TROWS, j0:j0 + nb, :],
                                         o_ps[0:TROWS, 0:nb, :],
                                         bias_sb[:, 0:nb, :])

                # output on the SWDGE path so input/output streams ride
                # different DMA queues; split so the kernel-tail transfer
                # (which nothing overlaps) is smaller
                for s0, sn in ((0, 21), (21, 20)):
                    nc.gpsimd.dma_start(
                        out=out_ext[:, t0 + s0:t0 + s0 + sn, :],
                        in_=o_t[0:TROWS, s0:s0 + sn, :])

    nc.compile()
    return nc


def _get_graph():
    if "nc" not in _CACHE:
        _CACHE["nc"] = _build_graph()
    return _CACHE["nc"]


def _host_prep(edge_index, gcn_W, gcn_b, mode=MODE):
    ei = np.asarray(edge_index).astype(np.int64)
    rows, cols = ei[0], ei[1]
    deg = np.bincount(cols, minlength=V).astype(np.float32) + 1.0  # + self loop
    dis = (1.0 / np.sqrt(deg)).astype(np.float32)
    M = np.zeros((V, V), np.float32)
    np.add.at(M, (cols, rows), dis[rows] * dis[cols])
    M[np.arange(V), np.arange(V)] += dis * dis
    m5t_pad = np.zeros((TROWS, C), np.float32)
    m5t_pad[:, :TROWS] = np.kron(np.eye(FPT, dtype=np.float32), M.T)
    bias_t = np.ascontiguousarray(
        np.broadcast_to(np.asarray(gcn_b, np.float32), (TROWS, JB, C)))
    return (m5t_pad.astype(ml_dtypes.bfloat16),
            np.asarray(gcn_W, np.float32).astype(ml_dtypes.bfloat16),
            bias_t)


def _pack(x):
    """(B, V, C) f32 -> per-core tile-major bf16 [NCORES, TROWS, NT, C]."""
    xr = np.asarray(x, np.float32).reshape(NCORES, ROWS, C)
    packed = np.zeros((NCORES, NT, TROWS, C), np.float32)
    packed[:, :FULL_T] = xr[:, :FULL_T * TROWS].reshape(NCORES, FULL_T, TROWS, C)
    packed[:, FULL_T, :TAIL_ROWS] = xr[:, FULL_T * TROWS:]
    return np.ascontiguousarray(
        packed.transpose(0, 2, 1, 3).astype(ml_dtypes.bfloat16))


def _unpack(outs):
    """[NCORES, TROWS, NT, C] -> (B, V, C)."""
    o = outs.transpose(0, 2, 1, 3)  # [NCORES, NT, TROWS, C]
    res = np.empty((NCORES, ROWS, C), np.float32)
    res[:, :FULL_T * TROWS] = o[:, :FULL_T].reshape(NCORES, FULL_T * TROWS, C)
    res[:, FULL_T * TROWS:] = o[:, FULL_T, :TAIL_ROWS]
    return res.reshape(B, V, C)


def kernel(x, edge_index, adj_matrix=None, aw_W=None, aw_b=None,
           gcn_W=None, gcn_b=None, **_unused):
    from concourse.bass_utils import run_bass_kernel_spmd

    m5t_h, w_h, bias_t = _host_prep(edge_index, gcn_W, gcn_b)
    xp = _pack(x)
    in_maps = [{"x": xp[i], "m5t": m5t_h, "w": w_h, "bias": bias_t}
               for i in range(NCORES)]
    res = run_bass_kernel_spmd(_get_graph(), in_maps,
                               core_ids=list(range(NCORES)))
    out = np.stack([r["out"] for r in res.results])
    return _unpack(out)


# revision 13
# speedup vs baseline: 2.0475x; 1.0252x over previous
"""AdaptiveGCNLayer Trainium2 kernel (8 NeuronCores, data-parallel over frames).

The reference module's adaptive-adjacency branch is dead code (its result is
never used).  Because edge_index is shared by every frame (offsets just shift
it per frame), the live computation collapses to

    out[f] = M @ x[f] @ gcn_W + gcn_b        for every frame f

with a single 25x25 normalized-adjacency matrix M (PyG GCNConv norm with
self-loops) computed on host from the 48 edges.

Sharding: frames are data-parallel across the 8 cores.  Each core's shard is
packed on host into tile-major layout [125 partitions, 205 tiles, 128 ch]
(5 frames = 125 rows per tile; the ragged tail is zero-padded) so every
HBM<->SBUF DMA is per-partition contiguous.

Device kernel (per core):
  - mm1: T1 = lhsT(x_tile).T @ (I5 (x) M^T)   -> (M5 @ X)^T in PSUM (no transposes)
  - copy T1 -> SBUF
  - mm2: O = lhsT(T1).T @ W                   -> natural row-major output in PSUM
  - DVE adds bias while copying PSUM -> SBUF
  - big contiguous HWDGE DMAs in and out

Two compute modes (KERNEL_MODE env): "bf16" casts x to bf16 on ACT/DVE and
runs bf16 matmuls; "f32r" feeds fp32 bits straight to the PE as float32r
with the moving free dim padded to 256 (full-rate per the cost model).
"""

import os
import numpy as np
import ml_dtypes

B, V, C = 8192, 25, 128
NCORES = 8
FRAMES_PER_CORE = B // NCORES          # 1024
ROWS = FRAMES_PER_CORE * V             # 25600
FPT = 5                                # frames per matmul tile
TROWS = FPT * V                        # 125 rows per tile
NT = 205                               # tiles per core (last one padded)
FULL_T = ROWS // TROWS                 # 204 full tiles
TAIL_ROWS = ROWS - FULL_T * TROWS      # 100
TPG = 41                               # tiles per DMA group
NGROUPS = NT // TPG                    # 5
JB = 4                                 # tiles per PSUM batch
MODE = os.environ.get("KERNEL_MODE", "bf16")
NF32R = 256                            # padded moving free dim for f32r

_CACHE = {}


def _build_graph(mode=MODE):
    import concourse.mybir as mybir
    import concourse.tile as tile
    from concourse import bacc

    f32 = mybir.dt.float32
    f32r = mybir.dt.float32r
    bf16 = mybir.dt.bfloat16

    nc = bacc.Bacc("TRN2", target_bir_lowering=False, debug=False,
                   num_devices=NCORES)

    # x arrives pre-cast to bf16 by the host (halves input traffic, no
    # on-chip cast stage needed)
    x_in = nc.declare_dram_parameter("x", [TROWS, NT, C], bf16, isOutput=False)
    m5t_in = nc.declare_dram_parameter("m5t", [TROWS, C], bf16, isOutput=False)
    w_in = nc.declare_dram_parameter("w", [C, C], bf16, isOutput=False)
    b_in = nc.declare_dram_parameter("bias", [TROWS, JB, C], f32, isOutput=False)
    out_ext = nc.declare_dram_parameter("out", [TROWS, NT, C], f32, isOutput=True)

    with tile.TileContext(nc) as tc:
        with (
            tc.tile_pool(name="consts", bufs=1) as consts,
            tc.tile_pool(name="xp", bufs=5) as xp,
            tc.tile_pool(name="op", bufs=2) as op_pool,
            tc.tile_pool(name="t1s", bufs=3) as t1sp,
            tc.tile_pool(name="t1psum", bufs=2, space=tile.bass.MemorySpace.PSUM) as t1pp,
            tc.tile_pool(name="opsum", bufs=2, space=tile.bass.MemorySpace.PSUM) as opp,
        ):
            m5t_sb = consts.tile([TROWS, C], bf16)
            w_sb = consts.tile([C, C], bf16)
            bias_sb = consts.tile([TROWS, JB, C], f32)
            nc.sync.dma_start(out=m5t_sb[:], in_=m5t_in[:])
            nc.sync.dma_start(out=w_sb[:], in_=w_in[:])
            nc.sync.dma_start(out=bias_sb[:], in_=b_in[:])

            for g in range(NGROUPS):
                t0 = g * TPG
                x_t = xp.tile([128, TPG, C], bf16, tag="x")
                # SWDGE loads (cheap async triggers), contiguous per
                # partition, bf16; sliced so the first matmuls unblock
                # before the whole group lands
                for s0, sn in ((0, 14), (14, 14), (28, 13)):
                    nc.gpsimd.dma_start(out=x_t[0:TROWS, s0:s0 + sn, :],
                                        in_=x_in[:, t0 + s0:t0 + s0 + sn, :])

                o_t = op_pool.tile([128, TPG, C], f32, tag="o")

                for j0 in range(0, TPG, JB):
                    nb = min(JB, TPG - j0)
                    t1p = t1pp.tile([128, JB, C], f32, tag="t1p")
                    for u in range(nb):
                        nc.tensor.matmul(t1p[:, u, :],
                                         lhsT=x_t[0:TROWS, j0 + u, :],
                                         rhs=m5t_sb[:, :],
                                         start=True, stop=True)
                    t1s = t1sp.tile([128, JB, C], bf16, tag="t1s")
                    nc.scalar.copy(t1s[:, 0:nb, :], t1p[:, 0:nb, :])
                    o_ps = opp.tile([128, JB, C], f32, tag="ops")
                    for u in range(nb):
                        nc.tensor.matmul(o_ps[:, u, :],
                                         lhsT=t1s[:, u, :],
                                         rhs=w_sb[:, :],
                                         start=True, stop=True)
                    nc.vector.tensor_add(o_t[0:TROWS, j0:j0 + nb, :],
                                         o_ps[0:TROWS, 0:nb, :],
                                         bias_sb[:, 0:nb, :])

                # output also SWDGE; split so the kernel-tail transfer
                # (which nothing overlaps) is smaller
                for s0, sn in ((0, 14), (14, 14), (28, 13)):
                    nc.gpsimd.dma_start(
                        out=out_ext[:, t0 + s0:t0 + s0 + sn, :],
                        in_=o_t[0:TROWS, s0:s0 + sn, :])

    nc.compile()
    return nc


def _get_graph():
    if "nc" not in _CACHE:
        _CACHE["nc"] = _build_graph()
    return _CACHE["nc"]


def _host_prep(edge_index, gcn_W, gcn_b, mode=MODE):
    ei = np.asarray(edge_index).astype(np.int64)
    rows, cols = ei[0], ei[1]
    deg = np.bincount(cols, minlength=V).astype(np.float32) + 1.0  # + self loop
    dis = (1.0 / np.sqrt(deg)).astype(np.float32)
    M = np.zeros((V, V), np.float32)
    np.add.at(M, (cols, rows), dis[rows] * dis[cols])
    M[np.arange(V), np.arange(V)] += dis * dis
    m5t_pad = np.zeros((TROWS, C), np.float32)
    m5t_pad[:, :TROWS] = np.kron(np.eye(FPT, dtype=np.float32), M.T)
    bias_t = np.ascontiguousarray(
        np.broadcast_to(np.asarray(gcn_b, np.float32), (TROWS, JB, C)))
    return (m5t_pad.astype(ml_dtypes.bfloat16),
            np.asarray(gcn_W, np.float32).astype(ml_dtypes.bfloat16),
            bias_t)


def _pack(x):
    """(B, V, C) f32 -> per-core tile-major bf16 [NCORES, TROWS, NT, C]."""
    xr = np.asarray(x, np.float32).reshape(NCORES, ROWS, C)
    packed = np.zeros((NCORES, NT, TROWS, C), np.float32)
    packed[:, :FULL_T] = xr[:, :FULL_T * TROWS].reshape(NCORES, FULL_T, TROWS, C)
    packed[:, FULL_T, :TAIL_ROWS] = xr[:, FULL_T * TROWS:]
    return np.ascontiguousarray(
        packed.transpose(0, 2, 1, 3).astype(ml_dtypes.bfloat16))


def _unpack(outs):
    """[NCORES, TROWS, NT, C] -> (B, V, C)."""
    o = outs.transpose(0, 2, 1, 3)  # [NCORES, NT, TROWS, C]
    res = np.empty((NCORES, ROWS, C), np.float32)
    res[:, :FULL_T * TROWS] = o[:, :FULL_T].reshape(NCORES, FULL_T * TROWS, C)
    res[:, FULL_T * TROWS:] = o[:, FULL_T, :TAIL_ROWS]
    return res.reshape(B, V, C)


def kernel(x, edge_index, adj_matrix=None, aw_W=None, aw_b=None,
           gcn_W=None, gcn_b=None, **_unused):
    from concourse.bass_utils import run_bass_kernel_spmd

    m5t_h, w_h, bias_t = _host_prep(edge_index, gcn_W, gcn_b)
    xp = _pack(x)
    in_maps = [{"x": xp[i], "m5t": m5t_h, "w": w_h, "bias": bias_t}
               for i in range(NCORES)]
    res = run_bass_kernel_spmd(_get_graph(), in_maps,
                               core_ids=list(range(NCORES)))
    out = np.stack([r["out"] for r in res.results])
    return _unpack(out)


# revision 16
# speedup vs baseline: 2.9335x; 1.4327x over previous
"""AdaptiveGCNLayer Trainium2 kernel (8 NeuronCores, data-parallel over frames).

The reference module's adaptive-adjacency branch is dead code (its result is
never used).  Because edge_index is shared by every frame (offsets just shift
it per frame), the live computation collapses to

    out[f] = M @ x[f] @ gcn_W + gcn_b        for every frame f

with a single 25x25 normalized-adjacency matrix M (PyG GCNConv norm with
self-loops) computed on host from the 48 edges.

Sharding: frames are data-parallel across the 8 cores.  Each core's shard is
packed on host into tile-major layout [125 partitions, 205 tiles, 128 ch]
(5 frames = 125 rows per tile; the ragged tail is zero-padded) so every
HBM<->SBUF DMA is per-partition contiguous.

Device kernel (per core):
  - mm1: T1 = lhsT(x_tile).T @ (I5 (x) M^T)   -> (M5 @ X)^T in PSUM (no transposes)
  - copy T1 -> SBUF
  - mm2: O = lhsT(T1).T @ W                   -> natural row-major output in PSUM
  - DVE adds bias while copying PSUM -> SBUF
  - big contiguous HWDGE DMAs in and out

Two compute modes (KERNEL_MODE env): "bf16" casts x to bf16 on ACT/DVE and
runs bf16 matmuls; "f32r" feeds fp32 bits straight to the PE as float32r
with the moving free dim padded to 256 (full-rate per the cost model).
"""

import os
import numpy as np
import ml_dtypes

B, V, C = 8192, 25, 128
NCORES = 8
FRAMES_PER_CORE = B // NCORES          # 1024
ROWS = FRAMES_PER_CORE * V             # 25600
FPT = 5                                # frames per matmul tile
TROWS = FPT * V                        # 125 rows per tile
NT = 205                               # tiles per core (last one padded)
FULL_T = ROWS // TROWS                 # 204 full tiles
TAIL_ROWS = ROWS - FULL_T * TROWS      # 100
TPG = 41                               # tiles per DMA group
NGROUPS = NT // TPG                    # 5
JB = 4                                 # tiles per PSUM batch
MODE = os.environ.get("KERNEL_MODE", "bf16")
NF32R = 256                            # padded moving free dim for f32r

_CACHE = {}


def _build_graph(mode=MODE):
    import concourse.mybir as mybir
    import concourse.tile as tile
    from concourse import bacc

    f32 = mybir.dt.float32
    f32r = mybir.dt.float32r
    bf16 = mybir.dt.bfloat16

    nc = bacc.Bacc("TRN2", target_bir_lowering=False, debug=False,
                   num_devices=NCORES)

    # x arrives pre-cast to bf16 by the host (halves input traffic, no
    # on-chip cast stage needed)
    x_in = nc.declare_dram_parameter("x", [TROWS, NT, C], bf16, isOutput=False)
    m5t_in = nc.declare_dram_parameter("m5t", [TROWS, C], bf16, isOutput=False)
    w_in = nc.declare_dram_parameter("w", [C, C], bf16, isOutput=False)
    b_in = nc.declare_dram_parameter("bias", [TROWS, JB, C], f32, isOutput=False)
    # output in bf16 (host upcasts back to f32) — halves output traffic
    out_ext = nc.declare_dram_parameter("out", [TROWS, NT, C], bf16, isOutput=True)

    with tile.TileContext(nc) as tc:
        with (
            tc.tile_pool(name="consts", bufs=1) as consts,
            tc.tile_pool(name="xp", bufs=5) as xp,
            tc.tile_pool(name="op", bufs=2) as op_pool,
            tc.tile_pool(name="t1s", bufs=3) as t1sp,
            tc.tile_pool(name="t1psum", bufs=2, space=tile.bass.MemorySpace.PSUM) as t1pp,
            tc.tile_pool(name="opsum", bufs=2, space=tile.bass.MemorySpace.PSUM) as opp,
        ):
            m5t_sb = consts.tile([TROWS, C], bf16)
            w_sb = consts.tile([C, C], bf16)
            bias_sb = consts.tile([TROWS, JB, C], f32)
            nc.sync.dma_start(out=m5t_sb[:], in_=m5t_in[:])
            nc.sync.dma_start(out=w_sb[:], in_=w_in[:])
            nc.sync.dma_start(out=bias_sb[:], in_=b_in[:])

            for g in range(NGROUPS):
                t0 = g * TPG
                x_t = xp.tile([128, TPG, C], bf16, tag="x")
                # SWDGE loads (cheap async triggers), contiguous per
                # partition, bf16; sliced so the first matmuls unblock
                # before the whole group lands
                for s0, sn in ((0, 14), (14, 14), (28, 13)):
                    nc.gpsimd.dma_start(out=x_t[0:TROWS, s0:s0 + sn, :],
                                        in_=x_in[:, t0 + s0:t0 + s0 + sn, :])

                o_t = op_pool.tile([128, TPG, C], bf16, tag="o")

                for j0 in range(0, TPG, JB):
                    nb = min(JB, TPG - j0)
                    t1p = t1pp.tile([128, JB, C], f32, tag="t1p")
                    for u in range(nb):
                        nc.tensor.matmul(t1p[:, u, :],
                                         lhsT=x_t[0:TROWS, j0 + u, :],
                                         rhs=m5t_sb[:, :],
                                         start=True, stop=True)
                    t1s = t1sp.tile([128, JB, C], bf16, tag="t1s")
                    nc.scalar.copy(t1s[:, 0:nb, :], t1p[:, 0:nb, :])
                    o_ps = opp.tile([128, JB, C], f32, tag="ops")
                    for u in range(nb):
                        nc.tensor.matmul(o_ps[:, u, :],
                                         lhsT=t1s[:, u, :],
                                         rhs=w_sb[:, :],
                                         start=True, stop=True)
                    nc.vector.tensor_add(o_t[0:TROWS, j0:j0 + nb, :],
                                         o_ps[0:TROWS, 0:nb, :],
                                         bias_sb[:, 0:nb, :])

                # output also SWDGE; split so the kernel-tail transfer
                # (which nothing overlaps) is smaller
                for s0, sn in ((0, 14), (14, 14), (28, 13)):
                    nc.gpsimd.dma_start(
                        out=out_ext[:, t0 + s0:t0 + s0 + sn, :],
                        in_=o_t[0:TROWS, s0:s0 + sn, :])

    nc.compile()
    return nc


def _get_graph():
    if "nc" not in _CACHE:
        _CACHE["nc"] = _build_graph()
    return _CACHE["nc"]


def _host_prep(edge_index, gcn_W, gcn_b, mode=MODE):
    ei = np.asarray(edge_index).astype(np.int64)
    rows, cols = ei[0], ei[1]
    deg = np.bincount(cols, minlength=V).astype(np.float32) + 1.0  # + self loop
    dis = (1.0 / np.sqrt(deg)).astype(np.float32)
    M = np.zeros((V, V), np.float32)
    np.add.at(M, (cols, rows), dis[rows] * dis[cols])
    M[np.arange(V), np.arange(V)] += dis * dis
    m5t_pad = np.zeros((TROWS, C), np.float32)
    m5t_pad[:, :TROWS] = np.kron(np.eye(FPT, dtype=np.float32), M.T)
    bias_t = np.ascontiguousarray(
        np.broadcast_to(np.asarray(gcn_b, np.float32), (TROWS, JB, C)))
    return (m5t_pad.astype(ml_dtypes.bfloat16),
            np.asarray(gcn_W, np.float32).astype(ml_dtypes.bfloat16),
            bias_t)


def _pack(x):
    """(B, V, C) f32 -> per-core tile-major bf16 [NCORES, TROWS, NT, C]."""
    xr = np.asarray(x, np.float32).reshape(NCORES, ROWS, C)
    packed = np.zeros((NCORES, NT, TROWS, C), np.float32)
    packed[:, :FULL_T] = xr[:, :FULL_T * TROWS].reshape(NCORES, FULL_T, TROWS, C)
    packed[:, FULL_T, :TAIL_ROWS] = xr[:, FULL_T * TROWS:]
    return np.ascontiguousarray(
        packed.transpose(0, 2, 1, 3).astype(ml_dtypes.bfloat16))


def _unpack(outs):
    """[NCORES, TROWS, NT, C] (bf16) -> (B, V, C) f32."""
    o = outs.transpose(0, 2, 1, 3).astype(np.float32)  # [NCORES, NT, TROWS, C]
    res = np.empty((NCORES, ROWS, C), np.float32)
    res[:, :FULL_T * TROWS] = o[:, :FULL_T].reshape(NCORES, FULL_T * TROWS, C)
    res[:, FULL_T * TROWS:] = o[:, FULL_T, :TAIL_ROWS]
    return res.reshape(B, V, C)


def kernel(x, edge_index, adj_matrix=None, aw_W=None, aw_b=None,
           gcn_W=None, gcn_b=None, **_unused):
    from concourse.bass_utils import run_bass_kernel_spmd

    m5t_h, w_h, bias_t = _host_prep(edge_index, gcn_W, gcn_b)
    xp = _pack(x)
    in_maps = [{"x": xp[i], "m5t": m5t_h, "w": w_h, "bias": bias_t}
               for i in range(NCORES)]
    res = run_bass_kernel_spmd(_get_graph(), in_maps,
                               core_ids=list(range(NCORES)))
    out = np.stack([r["out"] for r in res.results])
    return _unpack(out)
